# revision 13
# baseline (speedup 1.0000x reference)
"""Trainium2 Bass kernel for nn_CustomMatrixMultiplication.

Computes out[b, m] = sum_{n,p} m1[b, n, m] * m2[b, p, n]
              = sum_n m1[b, n, m] * s[b, n],   s[b, n] = sum_p m2[b, p, n]

Sharding: pure data parallel over batch B=64 across 8 NeuronCores
(8 batches per core).

The kernel is HBM-bound (64MB/core), so it is built as two back-to-back
STREAMING phases over one shared 8-buffer pool of 2MB chunks:
  phase A: stream all of m2 (16 x 2MB), s[b] = ones.T @ m2[b]
           (PE matmul, f32r, partition-dim reduction), then scatter
           s [1,1024] -> sT[b] [128,8] (tiny SBUF->SBUF DMA on gpsimd)
  phase B: stream all of m1, out[b] = sum_g sT[b][:,g].T @ m1tile[g]
           (PE matmul, f32r)
Chunks alternate between the two independent HWDGE rings (nc.sync /
nc.scalar), each a pure FIFO of 2MB loads (128 x 16KB descriptors) whose triggers only wait on matmuls that trail the
stream by ~1 chunk -- never on a cross-stage dependency chain. The
shared pool lets phase B's loads start draining while phase A is still
in flight, so the SDMA engines never idle between phases. The tiny sT
scatters ride gpsimd (SWDGE) where they cannot head-block a load ring;
their latency is hidden because sT[b] is only needed in phase B. The
last m1 batch is split into eighths so the final compute chases the
stream tail, and out-stores go on the sync ring (idle by then, and
HWDGE skips the SWDGE Q7 drain on the critical path). f32r (tf32-like,
~1e-4 rel) keeps the PE at 1 cycle/row for 512-wide moving operands;
accumulation is fp32 in PSUM.
"""

from contextlib import ExitStack

import numpy as np

import concourse.bacc as bacc
import concourse.mybir as mybir
import concourse.tile as tile
from concourse.bass_utils import run_bass_kernel_spmd

dt = mybir.dt

B, N, M, P = 64, 1024, 1024, 1024
NCORES = 8
BL = B // NCORES  # batches per core
H = 512           # matmul free-dim tile (fp32 moving-operand max)
R = 8             # row groups of 128 (1024 contraction rows / 128 partitions)

_cache = {}


def _build():
    nc = bacc.Bacc(None, target_bir_lowering=False)
    m1_d = nc.dram_tensor("matrix1", [BL, N, M], dt.float32r, kind="ExternalInput")
    m2_d = nc.dram_tensor("matrix2", [BL, P, N], dt.float32r, kind="ExternalInput")
    out_d = nc.dram_tensor("out", [BL, M], dt.float32, kind="ExternalOutput")

    with tile.TileContext(nc) as tc, ExitStack() as ctx:
        # one stream pool serves both phases: 4 x 4MB = 16MB rotating
        stream = ctx.enter_context(tc.tile_pool(name="stream", bufs=8))
        m1q = ctx.enter_context(tc.tile_pool(name="m1q", bufs=8))
        sp = ctx.enter_context(tc.tile_pool(name="sp", bufs=4))
        op = ctx.enter_context(tc.tile_pool(name="op", bufs=3))
        stp = ctx.enter_context(tc.tile_pool(name="stp", bufs=BL))
        const = ctx.enter_context(tc.tile_pool(name="const", bufs=1))
        psum = ctx.enter_context(tc.tile_pool(name="psum", bufs=3, space="PSUM"))

        ones_f32 = const.tile([128, 1], dt.float32)
        nc.vector.memset(ones_f32[:], 1.0)
        ones = const.tile([128, 1], dt.float32r)
        nc.vector.tensor_copy(ones[:], ones_f32[:])

        rings = [nc.sync, nc.scalar]
        sTs = [None] * BL
        m1ts = [None] * BL

        # ---- phase A: stream m2, compute s[b] -------------------------
        # row 8i+r -> partition i, free [r, n]; contiguous source
        # (128 descriptors x 32KB per 4MB chunk)
        R2 = R // 2
        for b in range(BL):
            m2_ap = m2_d[b].rearrange("(p r) n -> p r n", p=128)
            ps_s = psum.tile([1, N], dt.float32, tag="ps")
            for half in range(2):
                m2t = stream.tile([128, R2, N], dt.float32r, tag="st")
                rings[half].dma_start(
                    m2t[:], m2_ap[:, half * R2 : half * R2 + R2, :]
                )
                for h in range(N // H):
                    for r in range(half * R2, half * R2 + R2):
                        nc.tensor.matmul(
                            ps_s[0:1, H * h : H * (h + 1)],
                            ones[:],
                            m2t[:, r - half * R2, H * h : H * (h + 1)],
                            start=(r == 0),
                            stop=(r == R - 1),
                        )
            s_b = sp.tile([1, N], dt.float32r, tag="s")
            nc.vector.tensor_copy(s_b[:], ps_s[:])  # rounds to f32r
            # relayout sT[i, g] = s[8i + g] (4KB SBUF->SBUF) on gpsimd so
            # the two HWDGE rings stay pure big-load FIFOs
            sT = stp.tile([128, R], dt.float32r, tag="sT")
            nc.gpsimd.dma_start(sT[:], s_b[:])
            sTs[b] = sT

        # ---- phase B: stream m1, compute out[b] -----------------------
        def m1load(b, nparts):
            m1_ap = m1_d[b].rearrange("(p r) m -> p r m", p=128)
            rr = R // nparts
            pool, tag = (stream, "st") if nparts == 2 else (m1q, "m1q")
            parts = []
            for q in range(nparts):
                m1t = pool.tile([128, rr, M], dt.float32r, tag=tag)
                rings[q % 2].dma_start(
                    m1t[:], m1_ap[:, q * rr : (q + 1) * rr, :]
                )
                parts.append(m1t)
            m1ts[b] = parts

        def stage2(b):
            # out[m] = sum_g sum_i m1[8i+g, m] * s[8i+g]; per-h-slice
            # PSUM copies start as soon as that h's chain stops, then one
            # 4KB out DMA per batch
            sT, parts = sTs[b], m1ts[b]
            rr = R // len(parts)
            ps_o = psum.tile([1, M], dt.float32, tag="ps")
            o_b = op.tile([1, M], dt.float32, tag="o")
            for q, m1t in enumerate(parts):
                for h in range(M // H):
                    for g in range(q * rr, (q + 1) * rr):
                        nc.tensor.matmul(
                            ps_o[0:1, H * h : H * (h + 1)],
                            sT[:, g : g + 1],
                            m1t[:, g - q * rr, H * h : H * (h + 1)],
                            start=(g == 0),
                            stop=(g == R - 1),
                        )
                    if q == len(parts) - 1:
                        nc.vector.tensor_copy(
                            o_b[0:1, H * h : H * (h + 1)],
                            ps_o[0:1, H * h : H * (h + 1)],
                        )
            nc.sync.dma_start(out_d[b : b + 1, :], o_b[:])

        for b in range(BL):
            # last batch is split into eighths so the kernel tail is only
            # 512KB-drain -> 2 matmuls -> copy -> 4KB out DMA
            m1load(b, nparts=2 if b < BL - 1 else 8)
            stage2(b)

    nc.finalize()
    return nc


def _get_nc():
    if "nc" not in _cache:
        _cache["nc"] = _build()
    return _cache["nc"]


def kernel(matrix1, matrix2, _run_kwargs=None):
    m1 = np.ascontiguousarray(np.asarray(matrix1, dtype=np.float32))
    m2 = np.ascontiguousarray(np.asarray(matrix2, dtype=np.float32))
    assert m1.shape == (B, N, M) and m2.shape == (B, P, N)

    nc = _get_nc()
    in_maps = [
        {
            "matrix1": m1[i * BL : (i + 1) * BL],
            "matrix2": m2[i * BL : (i + 1) * BL],
        }
        for i in range(NCORES)
    ]
    res = run_bass_kernel_spmd(
        nc, in_maps, core_ids=list(range(NCORES)), **(_run_kwargs or {})
    )
    out = np.concatenate([res.results[i]["out"] for i in range(NCORES)], axis=0)
    if _run_kwargs:
        _cache["last_results"] = res
    return out


# revision 14
# speedup vs baseline: 1.0122x; 1.0122x over previous
"""Trainium2 Bass kernel for nn_CustomMatrixMultiplication.

Computes out[b, m] = sum_{n,p} m1[b, n, m] * m2[b, p, n]
              = sum_n m1[b, n, m] * s[b, n],   s[b, n] = sum_p m2[b, p, n]

Sharding: pure data parallel over batch B=64 across 8 NeuronCores
(8 batches per core).

The kernel is HBM-bound (64MB/core), so it is built as two back-to-back
STREAMING phases over one shared 8-buffer pool of 2MB chunks:
  phase A: stream all of m2 (16 x 2MB), s[b] = ones.T @ m2[b]
           (PE matmul, f32r, partition-dim reduction), then scatter
           s [1,1024] -> sT[b] [128,8] (tiny SBUF->SBUF DMA on gpsimd)
  phase B: stream all of m1, out[b] = sum_g sT[b][:,g].T @ m1tile[g]
           (PE matmul, f32r)
Chunks alternate between the two independent HWDGE rings (nc.sync /
nc.scalar), each a pure FIFO of 2MB loads (128 x 16KB descriptors) whose triggers only wait on matmuls that trail the
stream by ~1 chunk -- never on a cross-stage dependency chain. The
shared pool lets phase B's loads start draining while phase A is still
in flight, so the SDMA engines never idle between phases. The tiny sT
scatters ride gpsimd (SWDGE) where they cannot head-block a load ring;
their latency is hidden because sT[b] is only needed in phase B. The
last m1 batch is split into eighths so the final compute chases the
stream tail, and out-stores go on the sync ring (idle by then, and
HWDGE skips the SWDGE Q7 drain on the critical path). f32r (tf32-like,
~1e-4 rel) keeps the PE at 1 cycle/row for 512-wide moving operands;
accumulation is fp32 in PSUM.
"""

from contextlib import ExitStack

import numpy as np

import concourse.bacc as bacc
import concourse.mybir as mybir
import concourse.tile as tile
from concourse.bass_utils import run_bass_kernel_spmd

dt = mybir.dt

B, N, M, P = 64, 1024, 1024, 1024
NCORES = 8
BL = B // NCORES  # batches per core
H = 512           # matmul free-dim tile (fp32 moving-operand max)
R = 8             # row groups of 128 (1024 contraction rows / 128 partitions)

_cache = {}


def _build():
    nc = bacc.Bacc(None, target_bir_lowering=False)
    m1_d = nc.dram_tensor("matrix1", [BL, N, M], dt.float32r, kind="ExternalInput")
    m2_d = nc.dram_tensor("matrix2", [BL, P, N], dt.float32r, kind="ExternalInput")
    out_d = nc.dram_tensor("out", [BL, M], dt.float32, kind="ExternalOutput")

    with tile.TileContext(nc) as tc, ExitStack() as ctx:
        # one stream pool serves both phases: 4 x 4MB = 16MB rotating
        stream = ctx.enter_context(tc.tile_pool(name="stream", bufs=8))
        m1q = ctx.enter_context(tc.tile_pool(name="m1q", bufs=8))
        sp = ctx.enter_context(tc.tile_pool(name="sp", bufs=4))
        op = ctx.enter_context(tc.tile_pool(name="op", bufs=3))
        stp = ctx.enter_context(tc.tile_pool(name="stp", bufs=BL))
        const = ctx.enter_context(tc.tile_pool(name="const", bufs=1))
        psum = ctx.enter_context(tc.tile_pool(name="psum", bufs=3, space="PSUM"))

        ones_f32 = const.tile([128, 1], dt.float32)
        nc.vector.memset(ones_f32[:], 1.0)
        ones = const.tile([128, 1], dt.float32r)
        nc.vector.tensor_copy(ones[:], ones_f32[:])

        rings = [nc.sync, nc.scalar]
        sTs = [None] * BL
        m1ts = [None] * BL

        # ---- phase A: stream m2, compute s[b] -------------------------
        # row 8i+r -> partition i, free [r, n]; contiguous source
        # (128 descriptors x 32KB per 4MB chunk)
        R2 = R // 2
        for b in range(BL):
            m2_ap = m2_d[b].rearrange("(p r) n -> p r n", p=128)
            ps_s = psum.tile([1, N], dt.float32, tag="ps")
            for half in range(2):
                m2t = stream.tile([128, R2, N], dt.float32r, tag="st")
                rings[half].dma_start(
                    m2t[:], m2_ap[:, half * R2 : half * R2 + R2, :]
                )
                for h in range(N // H):
                    for r in range(half * R2, half * R2 + R2):
                        nc.tensor.matmul(
                            ps_s[0:1, H * h : H * (h + 1)],
                            ones[:],
                            m2t[:, r - half * R2, H * h : H * (h + 1)],
                            start=(r == 0),
                            stop=(r == R - 1),
                        )
            s_b = sp.tile([1, N], dt.float32r, tag="s")
            nc.vector.tensor_copy(s_b[:], ps_s[:])  # rounds to f32r
            # relayout sT[i, g] = s[8i + g] (4KB SBUF->SBUF) on gpsimd so
            # the two HWDGE rings stay pure big-load FIFOs
            sT = stp.tile([128, R], dt.float32r, tag="sT")
            nc.gpsimd.dma_start(sT[:], s_b[:])
            sTs[b] = sT

        # ---- phase B: stream m1, compute out[b] -----------------------
        def m1load(b, nparts):
            m1_ap = m1_d[b].rearrange("(p r) m -> p r m", p=128)
            rr = R // nparts
            pool, tag = (stream, "st") if nparts == 2 else (m1q, "m1q")
            parts = []
            for q in range(nparts):
                m1t = pool.tile([128, rr, M], dt.float32r, tag=tag)
                rings[q % 2].dma_start(
                    m1t[:], m1_ap[:, q * rr : (q + 1) * rr, :]
                )
                parts.append(m1t)
            m1ts[b] = parts

        def stage2(b):
            # out[m] = sum_g sum_i m1[8i+g, m] * s[8i+g]; per-h-slice
            # PSUM copies start as soon as that h's chain stops, then one
            # 4KB out DMA per batch
            sT, parts = sTs[b], m1ts[b]
            rr = R // len(parts)
            ps_o = psum.tile([1, M], dt.float32, tag="ps")
            o_b = op.tile([1, M], dt.float32, tag="o")
            for q, m1t in enumerate(parts):
                for h in range(M // H):
                    for g in range(q * rr, (q + 1) * rr):
                        nc.tensor.matmul(
                            ps_o[0:1, H * h : H * (h + 1)],
                            sT[:, g : g + 1],
                            m1t[:, g - q * rr, H * h : H * (h + 1)],
                            start=(g == 0),
                            stop=(g == R - 1),
                        )
                    if q == len(parts) - 1:
                        nc.vector.tensor_copy(
                            o_b[0:1, H * h : H * (h + 1)],
                            ps_o[0:1, H * h : H * (h + 1)],
                        )
            # out-stores ride gpsimd so they never occupy a load ring's
            # trigger-depth slot; the final one goes on sync (idle by
            # then) to skip the SWDGE Q7 drain on the critical path
            eng = nc.sync if b == BL - 1 else nc.gpsimd
            eng.dma_start(out_d[b : b + 1, :], o_b[:])

        for b in range(BL):
            # last batch is split into eighths so the kernel tail is only
            # 512KB-drain -> 2 matmuls -> copy -> 4KB out DMA
            m1load(b, nparts=2 if b < BL - 1 else 8)
            stage2(b)

    nc.finalize()
    return nc


def _get_nc():
    if "nc" not in _cache:
        _cache["nc"] = _build()
    return _cache["nc"]


def kernel(matrix1, matrix2, _run_kwargs=None):
    m1 = np.ascontiguousarray(np.asarray(matrix1, dtype=np.float32))
    m2 = np.ascontiguousarray(np.asarray(matrix2, dtype=np.float32))
    assert m1.shape == (B, N, M) and m2.shape == (B, P, N)

    nc = _get_nc()
    in_maps = [
        {
            "matrix1": m1[i * BL : (i + 1) * BL],
            "matrix2": m2[i * BL : (i + 1) * BL],
        }
        for i in range(NCORES)
    ]
    res = run_bass_kernel_spmd(
        nc, in_maps, core_ids=list(range(NCORES)), **(_run_kwargs or {})
    )
    out = np.concatenate([res.results[i]["out"] for i in range(NCORES)], axis=0)
    if _run_kwargs:
        _cache["last_results"] = res
    return out
